# revision 2
# baseline (speedup 1.0000x reference)
"""Trainium2 Bass kernel for nn_Attention (B=2, S=2048, D=1024, H=16) — v3.

Sharding: 8 cores = 2 batches x 4 head-groups (4 heads each), Megatron-style.

v3 = v2 (bf16 PE paths, batched prefetched DMAs, one-chunk-skewed ACT-bound
attention stream, approx-reciprocal + GPSIMD-broadcast normalize) PLUS full
cross-iteration software pipelining: the projection prelude of logical
iteration t+1 is emitted as fine-grained units popped into the PE slack of
iteration t's attention stream, with A/B-alternating activation buffers
(qTd/kTd/v_sb). One For_i body holds TWO logical iterations (A then B), so
`reps` must be even; the slope per requested rep stays one logical kernel.
"""

import numpy as np
import ml_dtypes

import concourse.bass as bass
import concourse.mybir as mybir
import concourse.tile as tile
from concourse import bacc
from concourse.bass_utils import run_bass_kernel_spmd

B, S, D = 2, 2048, 1024
H, HD = 16, 64
G = 4
GH = H // G
F = GH * HD
P = 128
KT = D // P
NS = S // 512
NC = S // P
BF = mybir.dt.bfloat16
FR = mybir.dt.float32r
F32 = mybir.dt.float32
EXP = mybir.ActivationFunctionType.Exp

_CACHED = None


def _build(reps=None):
    import contextlib

    nc = bacc.Bacc("TRN2", target_bir_lowering=False, debug=False, num_devices=8)

    xq = nc.dram_tensor("xq", [D, S], BF, kind="ExternalInput").ap()
    xk = nc.dram_tensor("xk", [D, S], BF, kind="ExternalInput").ap()
    xv = nc.dram_tensor("xv", [D, S], BF, kind="ExternalInput").ap()
    wq = nc.dram_tensor("wq", [D, F], BF, kind="ExternalInput").ap()
    wk = nc.dram_tensor("wk", [D, F], BF, kind="ExternalInput").ap()
    wv = nc.dram_tensor("wv", [D, F], BF, kind="ExternalInput").ap()
    wo = nc.dram_tensor("wo", [F, D], BF, kind="ExternalInput").ap()
    bq = nc.dram_tensor("bq", [P, F // P], F32, kind="ExternalInput").ap()
    bk = nc.dram_tensor("bk", [P, F // P], F32, kind="ExternalInput").ap()
    ot = nc.dram_tensor("ot", [D, S], BF, kind="ExternalOutput").ap()

    xq_r = xq.rearrange("(ko p) s -> p ko s", p=P)
    xk_r = xk.rearrange("(ko p) s -> p ko s", p=P)
    xv_r = xv.rearrange("(ko p) s -> p ko s", p=P)
    wq_r = wq.rearrange("(ko p) f -> p ko f", p=P)
    wk_r = wk.rearrange("(ko p) f -> p ko f", p=P)
    wv_r = wv.rearrange("(ko p) f -> p ko f", p=P)
    wo_r = wo.rearrange("(ko p) f -> p ko f", p=P)
    ot_r = ot.rearrange("(fo p) s -> p fo s", p=P)

    with tile.TileContext(nc) as tc:
        with (
            tc.tile_pool(name="wpool", bufs=1) as wpool,
            tc.tile_pool(name="xpool", bufs=1) as xpool,
            tc.tile_pool(name="apool", bufs=1) as apool,
            tc.tile_pool(name="epool", bufs=2) as epool,
            tc.tile_pool(name="rpool", bufs=2) as rpool,
            tc.tile_pool(name="opool", bufs=2) as opool,
            tc.tile_pool(name="ps_s", bufs=2, space="PSUM") as ps_s,
            tc.tile_pool(name="ps_o", bufs=2, space="PSUM") as ps_o,
            tc.tile_pool(name="ps_m", bufs=2, space="PSUM") as ps_m,
        ):
            # ---- persistent tiles ----
            wq_sb = wpool.tile([P, KT, F], BF)
            wk_sb = wpool.tile([P, KT, F], BF)
            wv_sb = wpool.tile([P, KT, F], BF)
            wo_sb = wpool.tile([P, F // P, D], BF)
            bq_sb = wpool.tile([P, F // P], F32)
            bk_sb = wpool.tile([P, F // P], F32)

            xk_n = [xpool.tile([P, KT, 512], BF, name=f"xk{n}") for n in range(NS)]
            xq_n = [xpool.tile([P, KT, 512], BF, name=f"xq{n}") for n in range(NS)]
            xv_n = [xpool.tile([P, KT, 512], BF, name=f"xv{n}") for n in range(NS)]

            # A/B-alternating activation sets. qTd holds i-block b (512 wide)
            # at rows (b%2)*64, cols (b//2)*512 — no duplication; the row
            # parity lines up with the scores pair's tile positions. kTd is
            # row-duplicated (both row-packed stationaries need the same k).
            sets = []
            for sn in ("A", "B"):
                sets.append(
                    dict(
                        qTd=[
                            apool.tile([P, S // 2], BF, name=f"qTd{sn}{h}")
                            for h in range(GH)
                        ],
                        kTd=[
                            apool.tile([P, S], BF, name=f"kTd{sn}{h}")
                            for h in range(GH)
                        ],
                        v_sb=apool.tile([P, NC, GH, HD + 1], BF, name=f"v_sb{sn}"),
                    )
                )
            stack = [apool.tile([P, S], BF, name=f"stack{t}") for t in range(2)]

            onesp_f = wpool.tile([P, 1], F32)
            nc.vector.memset(onesp_f[:], 1.0)
            onesp_b = wpool.tile([P, 1], BF)
            nc.vector.tensor_copy(onesp_b[:], onesp_f[:])

            # warmup outside the loop: pins the exp-table load + HAM ramp
            warm_f = wpool.tile([P, 128], F32)
            nc.vector.memset(warm_f[:], 0.0)
            warm_r = wpool.tile([P, 128], BF)
            nc.vector.tensor_copy(warm_r[:], warm_f[:])
            wexp = wpool.tile([P, 128], F32)
            nc.scalar.activation(wexp[:], warm_f[:], EXP)
            wps = ps_m.tile([P, 512], F32, tag="m", name="warmps")
            for _w in range(24):
                nc.tensor.matmul(
                    wps[:, 0:128], warm_r[:], warm_r[:],
                    start=(_w == 0), stop=(_w == 23),
                )

            def issue_dmas():
                nc.sync.dma_start(wk_sb[:], wk_r)
                nc.sync.dma_start(bk_sb[:], bk)
                nc.sync.dma_start(xk_n[0][:], xk_r[:, :, 0:512])
                nc.sync.dma_start(wq_sb[:], wq_r)
                nc.sync.dma_start(bq_sb[:], bq)
                nc.sync.dma_start(xq_n[0][:], xq_r[:, :, 0:512])
                nc.sync.dma_start(xq_n[1][:], xq_r[:, :, 512:1024])
                nc.sync.dma_start(wv_sb[:], wv_r)
                nc.sync.dma_start(xv_n[0][:], xv_r[:, :, 0:512])
                for n in range(1, NS):
                    nc.sync.dma_start(xk_n[n][:], xk_r[:, :, n * 512 : (n + 1) * 512])
                    nc.sync.dma_start(xv_n[n][:], xv_r[:, :, n * 512 : (n + 1) * 512])
                for n in range(2, NS):
                    nc.sync.dma_start(xq_n[n][:], xq_r[:, :, n * 512 : (n + 1) * 512])
                nc.sync.dma_start(wo_sb[:], wo_r)

            # ---- projection unit generators (write set `st`) ----
            def proj_qk_units(st, which, n, t):
                w_sb = wk_sb if which == "k" else wq_sb
                b_sb = bk_sb if which == "k" else bq_sb
                x_n = (xk_n if which == "k" else xq_n)[n]
                dst = st["kTd" if which == "k" else "qTd"]
                state = {}

                def mm_unit(kpair):
                    if kpair == 0:
                        state["ps"] = ps_m.tile(
                            [P, 512], F32, tag="m", name=f"p{which}{n}{t}"
                        )
                    ps = state["ps"]
                    for k in (2 * kpair, 2 * kpair + 1):
                        nc.tensor.matmul(
                            ps[:],
                            w_sb[:, k, t * P : (t + 1) * P],
                            x_n[:, k, :],
                            start=(k == 0),
                            stop=(k == KT - 1),
                        )

                def tsp_unit():
                    ps = state["ps"]
                    for hh in range(2):
                        h = 2 * t + hh
                        r = hh * 64
                        if which == "k":
                            # kTd row-duplicated into both partition halves
                            for dup in range(2):
                                nc.vector.tensor_scalar_add(
                                    dst[h][
                                        dup * 64 : dup * 64 + 64,
                                        n * 512 : (n + 1) * 512,
                                    ],
                                    ps[r : r + 64, :],
                                    b_sb[r : r + 64, t : t + 1],
                                )
                        else:
                            # qTd: i-block n at rows (n%2)*64, cols (n//2)*512
                            nc.vector.tensor_scalar_add(
                                dst[h][
                                    (n % 2) * 64 : (n % 2) * 64 + 64,
                                    (n // 2) * 512 : (n // 2) * 512 + 512,
                                ],
                                ps[r : r + 64, :],
                                b_sb[r : r + 64, t : t + 1],
                            )

                return [lambda kp=kp: mm_unit(kp) for kp in range(4)] + [tsp_unit]

            def vproj_units(st, n, c):
                state = {}

                def mm_unit(kq):
                    if kq == 0:
                        state["ps"] = ps_m.tile([P, F], F32, tag="m", name=f"pv{n}{c}")
                    psv = state["ps"]
                    for k in (4 * kq, 4 * kq + 1, 4 * kq + 2, 4 * kq + 3):
                        nc.tensor.matmul(
                            psv[:],
                            xv_n[n][:, k, c * P : (c + 1) * P],
                            wv_sb[:, k, :],
                            start=(k == 0),
                            stop=(k == KT - 1),
                        )

                def copy_unit():
                    psv = state["ps"]
                    ch = n * 4 + c
                    nc.vector.tensor_copy(
                        st["v_sb"][:, ch, :, 0:HD],
                        psv.rearrange("p (h e) -> p h e", e=HD),
                    )
                    nc.vector.tensor_copy(
                        st["v_sb"][:, ch, :, HD : HD + 1],
                        onesp_b[:, 0:1, None].to_broadcast((P, GH, 1)),
                    )

                return [lambda kq=kq: mm_unit(kq) for kq in range(2)] + [copy_unit]

            def prelude_units(st):
                units = []
                units += proj_qk_units(st, "k", 0, 0)
                units += proj_qk_units(st, "k", 0, 1)
                units += proj_qk_units(st, "q", 0, 0)
                units += proj_qk_units(st, "q", 0, 1)
                units += proj_qk_units(st, "q", 1, 0)
                units += proj_qk_units(st, "q", 1, 1)
                for c in range(4):
                    units += vproj_units(st, 0, c)
                for n in range(1, NS):
                    units += proj_qk_units(st, "k", n, 0)
                    units += proj_qk_units(st, "k", n, 1)
                    for c in range(4):
                        units += vproj_units(st, n, c)
                for n in (2, 3):
                    units += proj_qk_units(st, "q", n, 0)
                    units += proj_qk_units(st, "q", n, 1)
                return units

            def proj_unit_mm(fb, n):
                pf = ps_m.tile([P, 512], F32, tag="m", name=f"pf{fb}_{n}")
                for kk in range(F // P):
                    nc.tensor.matmul(
                        pf[:],
                        wo_sb[:, kk, fb * P : (fb + 1) * P],
                        stack[kk][:, n * 512 : (n + 1) * 512],
                        start=(kk == 0),
                        stop=(kk == F // P - 1),
                    )
                ob = opool.tile([P, 512], BF, tag="ob")
                nc.vector.tensor_copy(ob[:], pf[:])
                nc.scalar.dma_start(ot_r[:, fb, n * 512 : (n + 1) * 512], ob[:])

            # ---- one logical iteration: attention+output on set `st`,
            # draining `pending` units into the PE slack ----
            def stage2_3(st, pending):
                chunks = [
                    (half, h, j)
                    for half in range(2)
                    for h in range(GH)
                    for j in range(NC)
                ]
                qTd, kTd, v_sb = st["qTd"], st["kTd"], st["v_sb"]

                def scores(c):
                    half, h, j = c
                    ss = ps_s.tile([P, 1024], F32, tag="ss", name="ss")
                    for nn in range(2):
                        rb = nn * 64
                        # i-block 2*half+nn lives at qTd rows nn*64 (row
                        # parity == tile position), cols half*512
                        nc.tensor.matmul(
                            ss[:, nn * 512 : (nn + 1) * 512],
                            kTd[h][rb : rb + 64, j * P : (j + 1) * P],
                            qTd[h][rb : rb + 64, half * 512 : (half + 1) * 512],
                            start=True,
                            stop=True,
                            tile_position=(rb, 0),
                        )
                    eb = epool.tile([P, 1024], BF, tag="eb")
                    nc.scalar.activation(eb[:], ss[:], EXP)
                    return eb

                oacc_box = [None]
                eb_next = scores(chunks[0])
                for ci, c in enumerate(chunks):
                    half, h, j = c
                    i0 = half * 1024
                    t, r = h // 2, (h % 2) * 64
                    eb_c = eb_next
                    if ci + 1 < len(chunks):
                        eb_next = scores(chunks[ci + 1])
                    if j == 0:
                        oacc_box[0] = [
                            ps_o.tile([HD + 1, 512], F32, tag="oacc", name=f"oa{_n}")
                            for _n in range(2)
                        ]
                    oacc = oacc_box[0]
                    for nn in range(2):
                        nc.tensor.matmul(
                            oacc[nn][:],
                            v_sb[:, j, h, :],
                            eb_c[:, nn * 512 : (nn + 1) * 512],
                            start=(j == 0),
                            stop=(j == NC - 1),
                        )
                    # drain deferred units; drain harder as the end nears
                    remaining = len(chunks) - ci
                    if pending:
                        pending.pop(0)()
                    while pending and len(pending) > remaining:
                        pending.pop(0)()
                    if j == NC - 1:
                        for nn in range(2):
                            ocp = rpool.tile([HD + 1, 512], F32, tag="ocp")
                            nc.vector.tensor_copy(ocp[:], oacc[nn][:])
                            zt = rpool.tile([1, 512], F32, tag="zt")
                            # ACT HWDGE queue: tiny, never behind the 12MB
                            # input prefetch on the SP queue
                            nc.scalar.dma_start(zt[:], ocp[HD : HD + 1, :])

                            def norm_unit(t=t, r=r, i0=i0, nn=nn, ocp=ocp, zt=zt):
                                rec = rpool.tile([1, 512], F32, tag="rec")
                                nc.vector.reciprocal_approx_fast(rec[:], zt[:])
                                zb = rpool.tile([64, 512], F32, tag="bcast")
                                nc.gpsimd.partition_broadcast(zb[:], rec[:])
                                nc.vector.tensor_mul(
                                    stack[t][
                                        r : r + 64, i0 + nn * 512 : i0 + (nn + 1) * 512
                                    ],
                                    ocp[0:HD, :],
                                    zb[:],
                                )

                            # norms jump the queue: they release the
                            # 2-deep ocp/zt rotation within a couple chunks
                            pending.insert(0, norm_unit)
                        if h == GH - 1:
                            for nn2 in range(2):
                                for fb in range(D // P):
                                    pending.append(
                                        lambda fb=fb, n=2 * half + nn2: proj_unit_mm(
                                            fb, n
                                        )
                                    )
                while pending:
                    pending.pop(0)()

            # ---- prologue: fill set A for logical iteration 0 ----
            issue_dmas()
            for u in prelude_units(sets[0]):
                u()

            if reps:
                assert reps % 2 == 0, "reps must be even (2 logical iters per body)"
                with tc.For_i(
                    0,
                    reps // 2,
                    1,
                    hint_engines=(
                        mybir.EngineType.PE,
                        mybir.EngineType.DVE,
                        mybir.EngineType.Activation,
                        mybir.EngineType.SP,
                        mybir.EngineType.Pool,
                    ),
                ):
                    for cur, nxt in ((0, 1), (1, 0)):
                        issue_dmas()
                        pending = prelude_units(sets[nxt])
                        stage2_3(sets[cur], pending)
            else:
                stage2_3(sets[0], [])

    nc.compile()
    return nc


def get_nc():
    global _CACHED
    if _CACHED is None:
        _CACHED = _build()
    return _CACHED


def make_in_maps(query, key, value, Wq, bq, Wk, bk, Wv, bv, Wo, bo):
    bf16 = lambda a: np.ascontiguousarray(np.asarray(a, dtype=np.float32)).astype(
        ml_dtypes.bfloat16
    )
    f32 = lambda a: np.ascontiguousarray(np.asarray(a, dtype=np.float32))
    xq_b = [bf16(np.asarray(query)[b].T) for b in range(B)]
    xk_b = [bf16(np.asarray(key)[b].T) for b in range(B)]
    xv_b = [bf16(np.asarray(value)[b].T) for b in range(B)]
    Wq_, Wk_, Wv_, Wo_ = (np.asarray(w) for w in (Wq, Wk, Wv, Wo))
    bq_, bk_ = np.asarray(bq), np.asarray(bk)
    per_g = []
    for g in range(G):
        gs = slice(F * g, F * (g + 1))
        per_g.append(
            {
                "wq": bf16(Wq_[gs, :].T),
                "wk": bf16(Wk_[gs, :].T),
                "wv": bf16(Wv_[gs, :].T),
                "wo": bf16(Wo_[:, gs].T),
                "bq": f32(bq_[gs].reshape(F // P, P).T),
                "bk": f32(bk_[gs].reshape(F // P, P).T),
            }
        )
    in_maps = []
    for c in range(8):
        b, g = divmod(c, 4)
        in_maps.append({"xq": xq_b[b], "xk": xk_b[b], "xv": xv_b[b], **per_g[g]})
    return in_maps


def kernel(query, key, value, Wq, bq, Wk, bk, Wv, bv, Wo, bo):
    nc = get_nc()
    in_maps = make_in_maps(query, key, value, Wq, bq, Wk, bk, Wv, bv, Wo, bo)
    res = run_bass_kernel_spmd(nc, in_maps, core_ids=list(range(8)))
    bias_total = (
        np.asarray(bo, dtype=np.float64)
        + np.asarray(Wo, dtype=np.float64) @ np.asarray(bv, dtype=np.float64)
    ).astype(np.float32)
    outs = []
    for b in range(B):
        acc = np.zeros((D, S), np.float32)
        for g in range(G):
            acc += np.asarray(res.results[G * b + g]["ot"], dtype=np.float32)
        outs.append(acc.T + bias_total[None, :])
    return np.stack(outs).astype(np.float32)


# revision 3
# speedup vs baseline: 1.0257x; 1.0257x over previous
"""Trainium2 Bass kernel for nn_Attention (B=2, S=2048, D=1024, H=16) — v3.

Sharding: 8 cores = 2 batches x 4 head-groups (4 heads each), Megatron-style:
column-parallel QKV projections, local attention over the group's heads,
row-parallel output projection; host reduces the 4 partial outputs per batch
and folds bv through the softmax into the output bias (bo + Wo@bv).

Key structure (measured 226.3us/iter on 8-core trn2, rel_l2 8.9e-3):
- All PE operand paths in bf16 (inputs/weights converted on host; q/k/v/e
  produced bf16 at their source engine). PSUM accumulation stays f32.
- Attention stream is ACT-bound (exp [128,1024] ~1.24us back-to-back incl.
  the ~0.38us access-latency bubble). The PE is STRICT FIFO, so the stream
  is software-pipelined with a one-chunk skew: scores_{c+1} issues before
  attnV_c, keeping independent work ahead of the exp_c wait.
- Full cross-iteration pipelining: the projection prelude of logical
  iteration t+1 is split into ~0.5us units popped into the PE slack of
  iteration t's attention chunks, with A/B-alternating qTd/kTd/v_sb buffers.
  One For_i body = 2 logical iterations; `reps` must be even.
- qTd holds i-block b at rows (b%2)*64, cols (b//2)*512 (no row duplication;
  parity matches the scores pair's tile positions). kTd IS row-duplicated
  (HW requires stationary/moving to share a partition range).
- softmax denominator rides the attn@V matmul as a 65th ones-column; 1/Z via
  reciprocal_approx_fast (DVE Reciprocal is ~8 cyc/elem = 3.6us/row — far
  too slow) + GPSIMD partition_broadcast. Both custom ops require
  partition-0-aligned inputs, hence the tiny Z-row DMA off partition 64.
- Batched input DMAs (one per tensor per 512-seq tile) all issued up front
  on the SP queue; outputs (bf16) + Z-rows on the ACT HWDGE queue.
- Warmup (exp-table load + HAM ramp) sits OUTSIDE the For_i loop so the
  2.7us PSEUDO_LOAD_ACT_FUNC_SET doesn't replay per iteration.
"""

import numpy as np
import ml_dtypes

import concourse.bass as bass
import concourse.mybir as mybir
import concourse.tile as tile
from concourse import bacc
from concourse.bass_utils import run_bass_kernel_spmd

B, S, D = 2, 2048, 1024
H, HD = 16, 64
G = 4
GH = H // G
F = GH * HD
P = 128
KT = D // P
NS = S // 512
NC = S // P
BF = mybir.dt.bfloat16
FR = mybir.dt.float32r
F32 = mybir.dt.float32
EXP = mybir.ActivationFunctionType.Exp

_CACHED = None


def _build(reps=None):
    import contextlib

    nc = bacc.Bacc("TRN2", target_bir_lowering=False, debug=False, num_devices=8)

    xq = nc.dram_tensor("xq", [D, S], BF, kind="ExternalInput").ap()
    xk = nc.dram_tensor("xk", [D, S], BF, kind="ExternalInput").ap()
    xv = nc.dram_tensor("xv", [D, S], BF, kind="ExternalInput").ap()
    wq = nc.dram_tensor("wq", [D, F], BF, kind="ExternalInput").ap()
    wk = nc.dram_tensor("wk", [D, F], BF, kind="ExternalInput").ap()
    wv = nc.dram_tensor("wv", [D, F], BF, kind="ExternalInput").ap()
    wo = nc.dram_tensor("wo", [F, D], BF, kind="ExternalInput").ap()
    bq = nc.dram_tensor("bq", [P, F // P], F32, kind="ExternalInput").ap()
    bk = nc.dram_tensor("bk", [P, F // P], F32, kind="ExternalInput").ap()
    ot = nc.dram_tensor("ot", [D, S], BF, kind="ExternalOutput").ap()

    xq_r = xq.rearrange("(ko p) s -> p ko s", p=P)
    xk_r = xk.rearrange("(ko p) s -> p ko s", p=P)
    xv_r = xv.rearrange("(ko p) s -> p ko s", p=P)
    wq_r = wq.rearrange("(ko p) f -> p ko f", p=P)
    wk_r = wk.rearrange("(ko p) f -> p ko f", p=P)
    wv_r = wv.rearrange("(ko p) f -> p ko f", p=P)
    wo_r = wo.rearrange("(ko p) f -> p ko f", p=P)
    ot_r = ot.rearrange("(fo p) s -> p fo s", p=P)

    with tile.TileContext(nc) as tc:
        with (
            tc.tile_pool(name="wpool", bufs=1) as wpool,
            tc.tile_pool(name="xpool", bufs=1) as xpool,
            tc.tile_pool(name="apool", bufs=1) as apool,
            tc.tile_pool(name="epool", bufs=2) as epool,
            tc.tile_pool(name="rpool", bufs=2) as rpool,
            tc.tile_pool(name="opool", bufs=2) as opool,
            tc.tile_pool(name="ps_s", bufs=2, space="PSUM") as ps_s,
            tc.tile_pool(name="ps_o", bufs=2, space="PSUM") as ps_o,
            tc.tile_pool(name="ps_m", bufs=2, space="PSUM") as ps_m,
        ):
            # ---- persistent tiles ----
            wq_sb = wpool.tile([P, KT, F], BF)
            wk_sb = wpool.tile([P, KT, F], BF)
            wv_sb = wpool.tile([P, KT, F], BF)
            wo_sb = wpool.tile([P, F // P, D], BF)
            bq_sb = wpool.tile([P, F // P], F32)
            bk_sb = wpool.tile([P, F // P], F32)

            xk_n = [xpool.tile([P, KT, 512], BF, name=f"xk{n}") for n in range(NS)]
            xq_n = [xpool.tile([P, KT, 512], BF, name=f"xq{n}") for n in range(NS)]
            xv_n = [xpool.tile([P, KT, 512], BF, name=f"xv{n}") for n in range(NS)]

            # A/B-alternating activation sets. qTd holds i-block b (512 wide)
            # at rows (b%2)*64, cols (b//2)*512 — no duplication; the row
            # parity lines up with the scores pair's tile positions. kTd is
            # row-duplicated (both row-packed stationaries need the same k).
            sets = []
            for sn in ("A", "B"):
                sets.append(
                    dict(
                        qTd=[
                            apool.tile([P, S // 2], BF, name=f"qTd{sn}{h}")
                            for h in range(GH)
                        ],
                        kTd=[
                            apool.tile([P, S], BF, name=f"kTd{sn}{h}")
                            for h in range(GH)
                        ],
                        v_sb=apool.tile([P, NC, GH, HD + 1], BF, name=f"v_sb{sn}"),
                    )
                )
            stack = [apool.tile([P, S], BF, name=f"stack{t}") for t in range(2)]

            onesp_f = wpool.tile([P, 1], F32)
            nc.vector.memset(onesp_f[:], 1.0)
            onesp_b = wpool.tile([P, 1], BF)
            nc.vector.tensor_copy(onesp_b[:], onesp_f[:])

            # warmup outside the loop: pins the exp-table load + HAM ramp
            warm_f = wpool.tile([P, 128], F32)
            nc.vector.memset(warm_f[:], 0.0)
            warm_r = wpool.tile([P, 128], BF)
            nc.vector.tensor_copy(warm_r[:], warm_f[:])
            wexp = wpool.tile([P, 128], F32)
            nc.scalar.activation(wexp[:], warm_f[:], EXP)
            wps = ps_m.tile([P, 512], F32, tag="m", name="warmps")
            for _w in range(24):
                nc.tensor.matmul(
                    wps[:, 0:128], warm_r[:], warm_r[:],
                    start=(_w == 0), stop=(_w == 23),
                )

            def issue_dmas():
                nc.sync.dma_start(wk_sb[:], wk_r)
                nc.sync.dma_start(bk_sb[:], bk)
                nc.sync.dma_start(xk_n[0][:], xk_r[:, :, 0:512])
                nc.sync.dma_start(wq_sb[:], wq_r)
                nc.sync.dma_start(bq_sb[:], bq)
                nc.sync.dma_start(xq_n[0][:], xq_r[:, :, 0:512])
                nc.sync.dma_start(xq_n[1][:], xq_r[:, :, 512:1024])
                nc.sync.dma_start(wv_sb[:], wv_r)
                nc.sync.dma_start(xv_n[0][:], xv_r[:, :, 0:512])
                for n in range(1, NS):
                    nc.sync.dma_start(xk_n[n][:], xk_r[:, :, n * 512 : (n + 1) * 512])
                    nc.sync.dma_start(xv_n[n][:], xv_r[:, :, n * 512 : (n + 1) * 512])
                for n in range(2, NS):
                    nc.sync.dma_start(xq_n[n][:], xq_r[:, :, n * 512 : (n + 1) * 512])
                nc.sync.dma_start(wo_sb[:], wo_r)

            # ---- projection unit generators (write set `st`) ----
            def proj_qk_units(st, which, n, t):
                w_sb = wk_sb if which == "k" else wq_sb
                b_sb = bk_sb if which == "k" else bq_sb
                x_n = (xk_n if which == "k" else xq_n)[n]
                dst = st["kTd" if which == "k" else "qTd"]
                state = {}

                def mm_unit(kpair):
                    if kpair == 0:
                        state["ps"] = ps_m.tile(
                            [P, 512], F32, tag="m", name=f"p{which}{n}{t}"
                        )
                    ps = state["ps"]
                    for k in (2 * kpair, 2 * kpair + 1):
                        nc.tensor.matmul(
                            ps[:],
                            w_sb[:, k, t * P : (t + 1) * P],
                            x_n[:, k, :],
                            start=(k == 0),
                            stop=(k == KT - 1),
                        )

                def tsp_unit():
                    ps = state["ps"]
                    for hh in range(2):
                        h = 2 * t + hh
                        r = hh * 64
                        if which == "k":
                            # kTd row-duplicated into both partition halves
                            for dup in range(2):
                                nc.vector.tensor_scalar_add(
                                    dst[h][
                                        dup * 64 : dup * 64 + 64,
                                        n * 512 : (n + 1) * 512,
                                    ],
                                    ps[r : r + 64, :],
                                    b_sb[r : r + 64, t : t + 1],
                                )
                        else:
                            # qTd: i-block n at rows (n%2)*64, cols (n//2)*512
                            nc.vector.tensor_scalar_add(
                                dst[h][
                                    (n % 2) * 64 : (n % 2) * 64 + 64,
                                    (n // 2) * 512 : (n // 2) * 512 + 512,
                                ],
                                ps[r : r + 64, :],
                                b_sb[r : r + 64, t : t + 1],
                            )

                return [lambda kp=kp: mm_unit(kp) for kp in range(4)] + [tsp_unit]

            def vproj_units(st, n, c):
                state = {}

                def mm_unit(kq):
                    if kq == 0:
                        state["ps"] = ps_m.tile([P, F], F32, tag="m", name=f"pv{n}{c}")
                    psv = state["ps"]
                    for k in (4 * kq, 4 * kq + 1, 4 * kq + 2, 4 * kq + 3):
                        nc.tensor.matmul(
                            psv[:],
                            xv_n[n][:, k, c * P : (c + 1) * P],
                            wv_sb[:, k, :],
                            start=(k == 0),
                            stop=(k == KT - 1),
                        )

                def copy_unit():
                    psv = state["ps"]
                    ch = n * 4 + c
                    nc.vector.tensor_copy(
                        st["v_sb"][:, ch, :, 0:HD],
                        psv.rearrange("p (h e) -> p h e", e=HD),
                    )
                    nc.vector.tensor_copy(
                        st["v_sb"][:, ch, :, HD : HD + 1],
                        onesp_b[:, 0:1, None].to_broadcast((P, GH, 1)),
                    )

                return [lambda kq=kq: mm_unit(kq) for kq in range(2)] + [copy_unit]

            def prelude_units(st):
                units = []
                units += proj_qk_units(st, "k", 0, 0)
                units += proj_qk_units(st, "k", 0, 1)
                units += proj_qk_units(st, "q", 0, 0)
                units += proj_qk_units(st, "q", 0, 1)
                units += proj_qk_units(st, "q", 1, 0)
                units += proj_qk_units(st, "q", 1, 1)
                for c in range(4):
                    units += vproj_units(st, 0, c)
                for n in range(1, NS):
                    units += proj_qk_units(st, "k", n, 0)
                    units += proj_qk_units(st, "k", n, 1)
                    for c in range(4):
                        units += vproj_units(st, n, c)
                for n in (2, 3):
                    units += proj_qk_units(st, "q", n, 0)
                    units += proj_qk_units(st, "q", n, 1)
                return units

            def proj_unit_mm(fb, n):
                pf = ps_m.tile([P, 512], F32, tag="m", name=f"pf{fb}_{n}")
                for kk in range(F // P):
                    nc.tensor.matmul(
                        pf[:],
                        wo_sb[:, kk, fb * P : (fb + 1) * P],
                        stack[kk][:, n * 512 : (n + 1) * 512],
                        start=(kk == 0),
                        stop=(kk == F // P - 1),
                    )
                ob = opool.tile([P, 512], BF, tag="ob")
                nc.vector.tensor_copy(ob[:], pf[:])
                nc.scalar.dma_start(ot_r[:, fb, n * 512 : (n + 1) * 512], ob[:])

            # ---- one logical iteration: attention+output on set `st`,
            # draining `pending` units into the PE slack ----
            def stage2_3(st, pending):
                chunks = [
                    (half, h, j)
                    for half in range(2)
                    for h in range(GH)
                    for j in range(NC)
                ]
                qTd, kTd, v_sb = st["qTd"], st["kTd"], st["v_sb"]

                def scores(c):
                    half, h, j = c
                    ss = ps_s.tile([P, 1024], F32, tag="ss", name="ss")
                    for nn in range(2):
                        rb = nn * 64
                        # i-block 2*half+nn lives at qTd rows nn*64 (row
                        # parity == tile position), cols half*512
                        nc.tensor.matmul(
                            ss[:, nn * 512 : (nn + 1) * 512],
                            kTd[h][rb : rb + 64, j * P : (j + 1) * P],
                            qTd[h][rb : rb + 64, half * 512 : (half + 1) * 512],
                            start=True,
                            stop=True,
                            tile_position=(rb, 0),
                        )
                    eb = epool.tile([P, 1024], BF, tag="eb")
                    nc.scalar.activation(eb[:], ss[:], EXP)
                    return eb

                oacc_box = [None]
                eb_next = scores(chunks[0])
                for ci, c in enumerate(chunks):
                    half, h, j = c
                    i0 = half * 1024
                    t, r = h // 2, (h % 2) * 64
                    eb_c = eb_next
                    if ci + 1 < len(chunks):
                        eb_next = scores(chunks[ci + 1])
                    if j == 0:
                        oacc_box[0] = [
                            ps_o.tile([HD + 1, 512], F32, tag="oacc", name=f"oa{_n}")
                            for _n in range(2)
                        ]
                    oacc = oacc_box[0]
                    for nn in range(2):
                        nc.tensor.matmul(
                            oacc[nn][:],
                            v_sb[:, j, h, :],
                            eb_c[:, nn * 512 : (nn + 1) * 512],
                            start=(j == 0),
                            stop=(j == NC - 1),
                        )
                    # drain deferred units; drain harder as the end nears
                    remaining = len(chunks) - ci
                    if pending:
                        pending.pop(0)()
                    while pending and len(pending) > remaining:
                        pending.pop(0)()
                    if j == NC - 1:
                        for nn in range(2):
                            ocp = rpool.tile([HD + 1, 512], F32, tag="ocp")
                            nc.vector.tensor_copy(ocp[:], oacc[nn][:])
                            zt = rpool.tile([1, 512], F32, tag="zt")
                            # ACT HWDGE queue: tiny, never behind the 12MB
                            # input prefetch on the SP queue
                            nc.scalar.dma_start(zt[:], ocp[HD : HD + 1, :])

                            def norm_unit(t=t, r=r, i0=i0, nn=nn, ocp=ocp, zt=zt):
                                rec = rpool.tile([1, 512], F32, tag="rec")
                                nc.vector.reciprocal_approx_fast(rec[:], zt[:])
                                zb = rpool.tile([64, 512], F32, tag="bcast")
                                nc.gpsimd.partition_broadcast(zb[:], rec[:])
                                nc.vector.tensor_mul(
                                    stack[t][
                                        r : r + 64, i0 + nn * 512 : i0 + (nn + 1) * 512
                                    ],
                                    ocp[0:HD, :],
                                    zb[:],
                                )

                            # norms jump the queue: they release the
                            # 2-deep ocp/zt rotation within a couple chunks
                            pending.insert(0, norm_unit)
                        if h == GH - 1:
                            for nn2 in range(2):
                                for fb in range(D // P):
                                    pending.append(
                                        lambda fb=fb, n=2 * half + nn2: proj_unit_mm(
                                            fb, n
                                        )
                                    )
                while pending:
                    pending.pop(0)()

            # ---- prologue: fill set A for logical iteration 0 ----
            issue_dmas()
            for u in prelude_units(sets[0]):
                u()

            if reps:
                assert reps % 2 == 0, "reps must be even (2 logical iters per body)"
                with tc.For_i(
                    0,
                    reps // 2,
                    1,
                    hint_engines=(
                        mybir.EngineType.PE,
                        mybir.EngineType.DVE,
                        mybir.EngineType.Activation,
                        mybir.EngineType.SP,
                        mybir.EngineType.Pool,
                    ),
                ):
                    for cur, nxt in ((0, 1), (1, 0)):
                        issue_dmas()
                        pending = prelude_units(sets[nxt])
                        stage2_3(sets[cur], pending)
            else:
                stage2_3(sets[0], [])

    nc.compile()
    return nc


def get_nc():
    global _CACHED
    if _CACHED is None:
        _CACHED = _build()
    return _CACHED


def make_in_maps(query, key, value, Wq, bq, Wk, bk, Wv, bv, Wo, bo):
    bf16 = lambda a: np.ascontiguousarray(np.asarray(a, dtype=np.float32)).astype(
        ml_dtypes.bfloat16
    )
    f32 = lambda a: np.ascontiguousarray(np.asarray(a, dtype=np.float32))
    xq_b = [bf16(np.asarray(query)[b].T) for b in range(B)]
    xk_b = [bf16(np.asarray(key)[b].T) for b in range(B)]
    xv_b = [bf16(np.asarray(value)[b].T) for b in range(B)]
    Wq_, Wk_, Wv_, Wo_ = (np.asarray(w) for w in (Wq, Wk, Wv, Wo))
    bq_, bk_ = np.asarray(bq), np.asarray(bk)
    per_g = []
    for g in range(G):
        gs = slice(F * g, F * (g + 1))
        per_g.append(
            {
                "wq": bf16(Wq_[gs, :].T),
                "wk": bf16(Wk_[gs, :].T),
                "wv": bf16(Wv_[gs, :].T),
                "wo": bf16(Wo_[:, gs].T),
                "bq": f32(bq_[gs].reshape(F // P, P).T),
                "bk": f32(bk_[gs].reshape(F // P, P).T),
            }
        )
    in_maps = []
    for c in range(8):
        b, g = divmod(c, 4)
        in_maps.append({"xq": xq_b[b], "xk": xk_b[b], "xv": xv_b[b], **per_g[g]})
    return in_maps


def kernel(query, key, value, Wq, bq, Wk, bk, Wv, bv, Wo, bo):
    nc = get_nc()
    in_maps = make_in_maps(query, key, value, Wq, bq, Wk, bk, Wv, bv, Wo, bo)
    res = run_bass_kernel_spmd(nc, in_maps, core_ids=list(range(8)))
    bias_total = (
        np.asarray(bo, dtype=np.float64)
        + np.asarray(Wo, dtype=np.float64) @ np.asarray(bv, dtype=np.float64)
    ).astype(np.float32)
    outs = []
    for b in range(B):
        acc = np.zeros((D, S), np.float32)
        for g in range(G):
            acc += np.asarray(res.results[G * b + g]["ot"], dtype=np.float32)
        outs.append(acc.T + bias_total[None, :])
    return np.stack(outs).astype(np.float32)


# revision 5
# speedup vs baseline: 1.0544x; 1.0280x over previous
"""Trainium2 Bass kernel for nn_Attention (B=2, S=2048, D=1024, H=16) — v6.

Sharding: 8 cores = 2 batches x 4 head-groups (4 heads each), Megatron-style:
column-parallel QKV projections, local attention over the group's heads,
row-parallel output projection; host reduces the 4 partial outputs per batch
and folds bv through the softmax into the output bias (bo + Wo@bv).

Key structure (measured ~220us/iter on 8-core trn2, rel_l2 ~9.2e-3):
- All PE operand paths in bf16 (inputs/weights converted on host; q/k/v/e
  produced bf16 at their source engine). PSUM accumulation stays f32.
- Attention stream is ACT-bound (exp [128,1024] ~1.24us back-to-back incl.
  the ~0.38us access-latency bubble). The PE is STRICT FIFO, so the stream
  is software-pipelined with a TWO-chunk skew: scores_{c+2} issues before
  attnV_c, so exp_c's completion sem is already satisfied when attnV_c
  reaches the PE queue head (needs 3 eb buffers for the WAR ordering).
- Full cross-iteration pipelining: the projection prelude of logical
  iteration t+1 is split into ~0.5us units popped into the PE slack of
  iteration t's attention chunks, with A/B-alternating qTd/kTd/v_sb buffers.
  One For_i body = 2 logical iterations; `reps` must be even.
- qTd holds i-block b at rows (b%2)*64, cols (b//2)*512 (no row duplication;
  parity matches the scores pair's tile positions). kTd IS row-duplicated
  (HW requires stationary/moving to share a partition range).
- softmax denominator rides the attn@V matmul as a 65th ones-column; 1/Z via
  reciprocal_approx_fast (DVE Reciprocal is ~8 cyc/elem = 3.6us/row — far
  too slow) + GPSIMD partition_broadcast. Both custom ops require
  partition-0-aligned inputs, hence the Z-row GPSIMD casting DMA (bf16->f32,
  exact) off partition 64; its norm unit pops ~3 chunks later so the SWDGE
  latency never blocks the DVE FIFO head.
- Batched input DMAs (one per tensor per 512-seq tile) all issued up front
  on the SP queue; outputs (bf16) on the ACT HWDGE queue.
- Warmup (exp-table load + HAM ramp) sits OUTSIDE the For_i loop so the
  2.7us PSEUDO_LOAD_ACT_FUNC_SET doesn't replay per iteration.
"""

import numpy as np
import ml_dtypes

import concourse.bass as bass
import concourse.mybir as mybir
import concourse.tile as tile
from concourse import bacc
from concourse.bass_utils import run_bass_kernel_spmd

B, S, D = 2, 2048, 1024
H, HD = 16, 64
G = 4
GH = H // G
F = GH * HD
P = 128
KT = D // P
NS = S // 512
NC = S // P
BF = mybir.dt.bfloat16
FR = mybir.dt.float32r
F32 = mybir.dt.float32
EXP = mybir.ActivationFunctionType.Exp

_CACHED = None


def _build(reps=None):
    import contextlib

    nc = bacc.Bacc("TRN2", target_bir_lowering=False, debug=False, num_devices=8)

    xq = nc.dram_tensor("xq", [D, S], BF, kind="ExternalInput").ap()
    xk = nc.dram_tensor("xk", [D, S], BF, kind="ExternalInput").ap()
    xv = nc.dram_tensor("xv", [D, S], BF, kind="ExternalInput").ap()
    wq = nc.dram_tensor("wq", [D, F], BF, kind="ExternalInput").ap()
    wk = nc.dram_tensor("wk", [D, F], BF, kind="ExternalInput").ap()
    wv = nc.dram_tensor("wv", [D, F], BF, kind="ExternalInput").ap()
    wo = nc.dram_tensor("wo", [F, D], BF, kind="ExternalInput").ap()
    bq = nc.dram_tensor("bq", [P, F // P], F32, kind="ExternalInput").ap()
    bk = nc.dram_tensor("bk", [P, F // P], F32, kind="ExternalInput").ap()
    ot = nc.dram_tensor("ot", [D, S], BF, kind="ExternalOutput").ap()

    xq_r = xq.rearrange("(ko p) s -> p ko s", p=P)
    xk_r = xk.rearrange("(ko p) s -> p ko s", p=P)
    xv_r = xv.rearrange("(ko p) s -> p ko s", p=P)
    wq_r = wq.rearrange("(ko p) f -> p ko f", p=P)
    wk_r = wk.rearrange("(ko p) f -> p ko f", p=P)
    wv_r = wv.rearrange("(ko p) f -> p ko f", p=P)
    wo_r = wo.rearrange("(ko p) f -> p ko f", p=P)
    ot_r = ot.rearrange("(fo p) s -> p fo s", p=P)

    with tile.TileContext(nc) as tc:
        with (
            tc.tile_pool(name="wpool", bufs=1) as wpool,
            tc.tile_pool(name="xpool", bufs=1) as xpool,
            tc.tile_pool(name="apool", bufs=1) as apool,
            tc.tile_pool(name="epool", bufs=3) as epool,
            tc.tile_pool(name="rpool", bufs=2) as rpool,
            tc.tile_pool(name="opool", bufs=2) as opool,
            tc.tile_pool(name="ps_s", bufs=2, space="PSUM") as ps_s,
            tc.tile_pool(name="ps_o", bufs=2, space="PSUM") as ps_o,
            tc.tile_pool(name="ps_m", bufs=2, space="PSUM") as ps_m,
        ):
            # ---- persistent tiles ----
            wq_sb = wpool.tile([P, KT, F], BF)
            wk_sb = wpool.tile([P, KT, F], BF)
            wv_sb = wpool.tile([P, KT, F], BF)
            wo_sb = wpool.tile([P, F // P, D], BF)
            bq_sb = wpool.tile([P, F // P], F32)
            bk_sb = wpool.tile([P, F // P], F32)

            xk_n = [xpool.tile([P, KT, 512], BF, name=f"xk{n}") for n in range(NS)]
            xq_n = [xpool.tile([P, KT, 512], BF, name=f"xq{n}") for n in range(NS)]
            xv_n = [xpool.tile([P, KT, 512], BF, name=f"xv{n}") for n in range(NS)]

            # A/B-alternating activation sets. qTd holds i-block b (512 wide)
            # at rows (b%2)*64, cols (b//2)*512 — no duplication; the row
            # parity lines up with the scores pair's tile positions. kTd is
            # row-duplicated (both row-packed stationaries need the same k).
            sets = []
            for sn in ("A", "B"):
                sets.append(
                    dict(
                        qTd=[
                            apool.tile([P, S // 2], BF, name=f"qTd{sn}{h}")
                            for h in range(GH)
                        ],
                        kTd=[
                            apool.tile([P, S], BF, name=f"kTd{sn}{h}")
                            for h in range(GH)
                        ],
                        v_sb=apool.tile([P, NC, GH, HD + 1], BF, name=f"v_sb{sn}"),
                    )
                )
            stack = [apool.tile([P, S], BF, name=f"stack{t}") for t in range(2)]

            onesp_f = wpool.tile([P, 1], F32)
            nc.vector.memset(onesp_f[:], 1.0)
            onesp_b = wpool.tile([P, 1], BF)
            nc.vector.tensor_copy(onesp_b[:], onesp_f[:])

            # warmup outside the loop: pins the exp-table load + HAM ramp
            warm_f = wpool.tile([P, 64], F32)
            nc.vector.memset(warm_f[:], 0.0)
            warm_r = wpool.tile([P, 64], BF)
            nc.vector.tensor_copy(warm_r[:], warm_f[:])
            wexp = wpool.tile([P, 64], F32)
            nc.scalar.activation(wexp[:], warm_f[:], EXP)
            wps = ps_m.tile([P, 512], F32, tag="m", name="warmps")
            for _w in range(24):
                nc.tensor.matmul(
                    wps[0:64, 0:64], warm_r[:], warm_r[:],
                    start=(_w == 0), stop=(_w == 23),
                )

            def issue_dmas():
                nc.sync.dma_start(wk_sb[:], wk_r)
                nc.sync.dma_start(bk_sb[:], bk)
                nc.sync.dma_start(xk_n[0][:], xk_r[:, :, 0:512])
                nc.sync.dma_start(wq_sb[:], wq_r)
                nc.sync.dma_start(bq_sb[:], bq)
                nc.sync.dma_start(xq_n[0][:], xq_r[:, :, 0:512])
                nc.sync.dma_start(xq_n[1][:], xq_r[:, :, 512:1024])
                nc.sync.dma_start(wv_sb[:], wv_r)
                nc.sync.dma_start(xv_n[0][:], xv_r[:, :, 0:512])
                for n in range(1, NS):
                    nc.sync.dma_start(xk_n[n][:], xk_r[:, :, n * 512 : (n + 1) * 512])
                    nc.sync.dma_start(xv_n[n][:], xv_r[:, :, n * 512 : (n + 1) * 512])
                for n in range(2, NS):
                    nc.sync.dma_start(xq_n[n][:], xq_r[:, :, n * 512 : (n + 1) * 512])
                nc.sync.dma_start(wo_sb[:], wo_r)

            # ---- projection unit generators (write set `st`) ----
            def proj_qk_units(st, which, n, t):
                w_sb = wk_sb if which == "k" else wq_sb
                b_sb = bk_sb if which == "k" else bq_sb
                x_n = (xk_n if which == "k" else xq_n)[n]
                dst = st["kTd" if which == "k" else "qTd"]
                state = {}

                def mm_unit(kpair):
                    if kpair == 0:
                        state["ps"] = ps_m.tile(
                            [P, 512], F32, tag="m", name=f"p{which}{n}{t}"
                        )
                    ps = state["ps"]
                    for k in (2 * kpair, 2 * kpair + 1):
                        nc.tensor.matmul(
                            ps[:],
                            w_sb[:, k, t * P : (t + 1) * P],
                            x_n[:, k, :],
                            start=(k == 0),
                            stop=(k == KT - 1),
                        )

                def tsp_unit():
                    ps = state["ps"]
                    for hh in range(2):
                        h = 2 * t + hh
                        r = hh * 64
                        if which == "k":
                            # kTd row-duplicated into both partition halves
                            for dup in range(2):
                                nc.vector.tensor_scalar_add(
                                    dst[h][
                                        dup * 64 : dup * 64 + 64,
                                        n * 512 : (n + 1) * 512,
                                    ],
                                    ps[r : r + 64, :],
                                    b_sb[r : r + 64, t : t + 1],
                                )
                        else:
                            # qTd: i-block n at rows (n%2)*64, cols (n//2)*512
                            nc.vector.tensor_scalar_add(
                                dst[h][
                                    (n % 2) * 64 : (n % 2) * 64 + 64,
                                    (n // 2) * 512 : (n // 2) * 512 + 512,
                                ],
                                ps[r : r + 64, :],
                                b_sb[r : r + 64, t : t + 1],
                            )

                return [lambda kp=kp: mm_unit(kp) for kp in range(4)] + [tsp_unit]

            def vproj_units(st, n, c):
                state = {}

                def mm_unit(kq):
                    if kq == 0:
                        state["ps"] = ps_m.tile([P, F], F32, tag="m", name=f"pv{n}{c}")
                    psv = state["ps"]
                    for k in (4 * kq, 4 * kq + 1, 4 * kq + 2, 4 * kq + 3):
                        nc.tensor.matmul(
                            psv[:],
                            xv_n[n][:, k, c * P : (c + 1) * P],
                            wv_sb[:, k, :],
                            start=(k == 0),
                            stop=(k == KT - 1),
                        )

                def copy_unit():
                    psv = state["ps"]
                    ch = n * 4 + c
                    nc.vector.tensor_copy(
                        st["v_sb"][:, ch, :, 0:HD],
                        psv.rearrange("p (h e) -> p h e", e=HD),
                    )
                    nc.vector.tensor_copy(
                        st["v_sb"][:, ch, :, HD : HD + 1],
                        onesp_b[:, 0:1, None].to_broadcast((P, GH, 1)),
                    )

                return [lambda kq=kq: mm_unit(kq) for kq in range(2)] + [copy_unit]

            def prelude_units(st):
                units = []
                units += proj_qk_units(st, "k", 0, 0)
                units += proj_qk_units(st, "k", 0, 1)
                units += proj_qk_units(st, "q", 0, 0)
                units += proj_qk_units(st, "q", 0, 1)
                units += proj_qk_units(st, "q", 1, 0)
                units += proj_qk_units(st, "q", 1, 1)
                for c in range(4):
                    units += vproj_units(st, 0, c)
                for n in range(1, NS):
                    units += proj_qk_units(st, "k", n, 0)
                    units += proj_qk_units(st, "k", n, 1)
                    for c in range(4):
                        units += vproj_units(st, n, c)
                for n in (2, 3):
                    units += proj_qk_units(st, "q", n, 0)
                    units += proj_qk_units(st, "q", n, 1)
                return units

            def proj_unit_mm(fb, n):
                pf = ps_m.tile([P, 512], F32, tag="m", name=f"pf{fb}_{n}")
                for kk in range(F // P):
                    nc.tensor.matmul(
                        pf[:],
                        wo_sb[:, kk, fb * P : (fb + 1) * P],
                        stack[kk][:, n * 512 : (n + 1) * 512],
                        start=(kk == 0),
                        stop=(kk == F // P - 1),
                    )
                ob = opool.tile([P, 512], BF, tag="ob")
                nc.vector.tensor_copy(ob[:], pf[:])
                nc.scalar.dma_start(ot_r[:, fb, n * 512 : (n + 1) * 512], ob[:])

            # ---- one logical iteration: attention+output on set `st`,
            # draining `pending` units into the PE slack ----
            def stage2_3(st, pending):
                chunks = [
                    (half, h, j)
                    for half in range(2)
                    for h in range(GH)
                    for j in range(NC)
                ]
                qTd, kTd, v_sb = st["qTd"], st["kTd"], st["v_sb"]

                def scores(c):
                    half, h, j = c
                    ss = ps_s.tile([P, 1024], F32, tag="ss", name="ss")
                    for nn in range(2):
                        rb = nn * 64
                        # i-block 2*half+nn lives at qTd rows nn*64 (row
                        # parity == tile position), cols half*512
                        nc.tensor.matmul(
                            ss[:, nn * 512 : (nn + 1) * 512],
                            kTd[h][rb : rb + 64, j * P : (j + 1) * P],
                            qTd[h][rb : rb + 64, half * 512 : (half + 1) * 512],
                            start=True,
                            stop=True,
                            tile_position=(rb, 0),
                        )
                    eb = epool.tile([P, 1024], BF, tag="eb")
                    nc.scalar.activation(eb[:], ss[:], EXP)
                    return eb

                oacc_box = [None]
                # two-chunk skew: scores_{c+2} issues before attnV_c, so the
                # exp_c completion sem is already satisfied when attnV_c
                # reaches the strict-FIFO PE queue head (needs 3 eb buffers)
                eb_q = [scores(chunks[0]), scores(chunks[1])]
                for ci, c in enumerate(chunks):
                    half, h, j = c
                    i0 = half * 1024
                    t, r = h // 2, (h % 2) * 64
                    eb_c = eb_q.pop(0)
                    if ci + 2 < len(chunks):
                        eb_q.append(scores(chunks[ci + 2]))
                    if j == 0:
                        oacc_box[0] = [
                            ps_o.tile([HD + 1, 512], F32, tag="oacc", name=f"oa{_n}")
                            for _n in range(2)
                        ]
                    oacc = oacc_box[0]
                    for nn in range(2):
                        nc.tensor.matmul(
                            oacc[nn][:],
                            v_sb[:, j, h, :],
                            eb_c[:, nn * 512 : (nn + 1) * 512],
                            start=(j == 0),
                            stop=(j == NC - 1),
                        )
                    # drain deferred units; drain harder as the end nears
                    remaining = len(chunks) - ci
                    if pending:
                        pending.pop(0)()
                    while pending and len(pending) > remaining:
                        pending.pop(0)()
                    if j == NC - 1:
                        for nn in range(2):
                            ocp = rpool.tile([HD + 1, 512], BF, tag="ocp")
                            nc.vector.tensor_copy(ocp[:], oacc[nn][:])
                            zt = rpool.tile([1, 512], F32, tag="zt")
                            # GPSIMD casting DMA (bf16->f32, exact): lands Z
                            # on partition 0 for the custom DVE op below
                            nc.gpsimd.dma_start(zt[:], ocp[HD : HD + 1, :])

                            def norm_unit(t=t, r=r, i0=i0, nn=nn, ocp=ocp, zt=zt):
                                rec = rpool.tile([1, 512], F32, tag="rec")
                                nc.vector.reciprocal_approx_fast(rec[:], zt[:])
                                zb = rpool.tile([64, 512], F32, tag="bcast")
                                nc.gpsimd.partition_broadcast(zb[:], rec[:])
                                nc.vector.tensor_mul(
                                    stack[t][
                                        r : r + 64, i0 + nn * 512 : i0 + (nn + 1) * 512
                                    ],
                                    ocp[0:HD, :],
                                    zb[:],
                                )

                            # norms near the queue front (released
                            # within a few chunks for the 2-deep ocp/zt
                            # rotation) but delayed ~3 chunks so the Z-row
                            # SWDGE cast has landed before the DVE pops it
                            pending.insert(min(3, len(pending)), norm_unit)
                        if h == GH - 1:
                            for nn2 in range(2):
                                for fb in range(D // P):
                                    pending.append(
                                        lambda fb=fb, n=2 * half + nn2: proj_unit_mm(
                                            fb, n
                                        )
                                    )
                while pending:
                    pending.pop(0)()

            # ---- prologue: fill set A for logical iteration 0 ----
            issue_dmas()
            for u in prelude_units(sets[0]):
                u()

            if reps:
                assert reps % 2 == 0, "reps must be even (2 logical iters per body)"
                with tc.For_i(
                    0,
                    reps // 2,
                    1,
                    hint_engines=(
                        mybir.EngineType.PE,
                        mybir.EngineType.DVE,
                        mybir.EngineType.Activation,
                        mybir.EngineType.SP,
                        mybir.EngineType.Pool,
                    ),
                ):
                    for cur, nxt in ((0, 1), (1, 0)):
                        issue_dmas()
                        pending = prelude_units(sets[nxt])
                        stage2_3(sets[cur], pending)
            else:
                stage2_3(sets[0], [])

    nc.compile()
    return nc


def get_nc():
    global _CACHED
    if _CACHED is None:
        _CACHED = _build()
    return _CACHED


def make_in_maps(query, key, value, Wq, bq, Wk, bk, Wv, bv, Wo, bo):
    bf16 = lambda a: np.ascontiguousarray(np.asarray(a, dtype=np.float32)).astype(
        ml_dtypes.bfloat16
    )
    f32 = lambda a: np.ascontiguousarray(np.asarray(a, dtype=np.float32))
    xq_b = [bf16(np.asarray(query)[b].T) for b in range(B)]
    xk_b = [bf16(np.asarray(key)[b].T) for b in range(B)]
    xv_b = [bf16(np.asarray(value)[b].T) for b in range(B)]
    Wq_, Wk_, Wv_, Wo_ = (np.asarray(w) for w in (Wq, Wk, Wv, Wo))
    bq_, bk_ = np.asarray(bq), np.asarray(bk)
    per_g = []
    for g in range(G):
        gs = slice(F * g, F * (g + 1))
        per_g.append(
            {
                "wq": bf16(Wq_[gs, :].T),
                "wk": bf16(Wk_[gs, :].T),
                "wv": bf16(Wv_[gs, :].T),
                "wo": bf16(Wo_[:, gs].T),
                "bq": f32(bq_[gs].reshape(F // P, P).T),
                "bk": f32(bk_[gs].reshape(F // P, P).T),
            }
        )
    in_maps = []
    for c in range(8):
        b, g = divmod(c, 4)
        in_maps.append({"xq": xq_b[b], "xk": xk_b[b], "xv": xv_b[b], **per_g[g]})
    return in_maps


def kernel(query, key, value, Wq, bq, Wk, bk, Wv, bv, Wo, bo):
    nc = get_nc()
    in_maps = make_in_maps(query, key, value, Wq, bq, Wk, bk, Wv, bv, Wo, bo)
    res = run_bass_kernel_spmd(nc, in_maps, core_ids=list(range(8)))
    bias_total = (
        np.asarray(bo, dtype=np.float64)
        + np.asarray(Wo, dtype=np.float64) @ np.asarray(bv, dtype=np.float64)
    ).astype(np.float32)
    outs = []
    for b in range(B):
        acc = np.zeros((D, S), np.float32)
        for g in range(G):
            acc += np.asarray(res.results[G * b + g]["ot"], dtype=np.float32)
        outs.append(acc.T + bias_total[None, :])
    return np.stack(outs).astype(np.float32)
